# revision 2
# baseline (speedup 1.0000x reference)
"""Trainium2 Bass kernel for nn_Mixture_24541443129646.

loss = 0.5*S_xx - sum_n log sum_k exp(cross[n,k] - musq[k]/2)   (+ N*C shift)

Transposed layout (k on partitions), data-parallel over N on 8 cores.
ACT-centric pipeline. Per 1024-col block j:
  PE:   psA(j%2) = mupt0^T @ xt_j ; psB = mupt1^T @ xt_j
        rowsum(j-1): o8^T @ e01 -> pR partitions 32*(j-1)%4 (col tile_position)
  ACT:  e0 = Exp(psA + bc0), e1 = Exp(psB + bc1)          (the clock, ~1.04us each)
  DVE:  e01 = e0 + e1 (bf16 perf mode)
v5: rowsums of 4 consecutive blocks pack into ONE [128,1024] pR tile at
partition groups 0/32/64/96, so the PSUM->SBUF copy + DRAM drain happen
once per 4 blocks and the pR reuse cycle has ~4 exp-periods of slack
(v4 lagged: per-block copy->rsMM->copy cycle was 2.7us > 2.36us period).
Tail: ONE reload rs_dram -> rsd [128,128], Ln+accum, out.
PSUM banks: pA bufs=2 (4) + pB bufs=1 (2) + pR bufs=1 (2) = 8.
Host: shard/transpose/cast x, fold prec into mu, S_xx, final scalar.
"""

import sys

sys.path.insert(0, "/opt/trn_rl_repo")

from contextlib import ExitStack

import numpy as np

import concourse.bass as bass
import concourse.tile as tile
from concourse import mybir
from concourse.bass_utils import run_bass_kernel_spmd

N, K, D = 131072, 256, 128
NCORES = 8
RPC = N // NCORES
CW = 1024
NCHUNK = RPC // CW
MMW = 512
CSHIFT = 40.0

F32 = mybir.dt.float32
BF16 = mybir.dt.bfloat16
ACTF = mybir.ActivationFunctionType

XT_LOADS = [1024, 1024, 2048, 4096, 4096, 4096]
assert sum(XT_LOADS) == RPC


def _strip_same_engine_waits(nc):
    """Remove waits on semaphores that are only ever incremented by EARLIER
    non-DMA instructions of the SAME engine. Engines execute their queues in
    order, so these waits are satisfied by construction; leaving them in
    blocks back-to-back issue pipelining (~150ns per exp on ACT).

    Safe here: no engine in this kernel reads data written by an earlier
    instruction of the same engine (all data deps are cross-engine and keep
    their waits)."""
    from collections import defaultdict

    all_ins = []
    for f in nc.m.functions:
        for bb in f.blocks:
            all_ins.extend(bb.instructions)

    STRIP_ENGINES = {
        mybir.EngineType.Activation,
        mybir.EngineType.PE,
        mybir.EngineType.DVE,
    }

    sem_ok = {}  # sem id -> owning engine (or None if disqualified)
    for ins in all_ins:
        si = ins.sync_info
        if si is None:
            continue
        is_dma = "DMA" in type(ins).__name__ or "Dma" in type(ins).__name__
        for u in si.on_update:
            sid = u.id
            if u.update_mode != "sem-inc" or is_dma:
                sem_ok[sid] = None
                continue
            if sid not in sem_ok:
                sem_ok[sid] = ins.engine
            elif sem_ok[sid] != ins.engine:
                sem_ok[sid] = None

    n_strip = 0
    cum = defaultdict(int)
    for ins in all_ins:
        si = ins.sync_info
        if si is None:
            continue
        if si.on_wait:
            keep = []
            for w in si.on_wait:
                if (
                    w.wait_mode == "sem-ge-imm"
                    and ins.engine in STRIP_ENGINES
                    and sem_ok.get(w.id) == ins.engine
                    and cum[w.id] >= w.wait_value
                ):
                    n_strip += 1
                else:
                    keep.append(w)
            si.on_wait = keep
        for u in si.on_update:
            if u.update_mode == "sem-inc":
                cum[u.id] += u.update_value
    return n_strip


def _split_excess_waits(nc, max_waits=1):
    import bass_rust

    n_fix = 0
    for f in nc.m.functions:
        for bb in f.blocks:
            insts = bb.instructions
            out_list = []
            changed = False
            for ins in insts:
                si = ins.sync_info
                if si is not None and len(si.on_wait) > max_waits:
                    waits = list(si.on_wait)
                    extra, keep = waits[:-max_waits], waits[-max_waits:]
                    for i in range(0, len(extra), max_waits):
                        nd = mybir.InstDrain(name=f"I-waitfix-{n_fix}", ins=[], outs=[])
                        n_fix += 1
                        nd.engine = ins.engine
                        nd.sync_info = bass_rust.SyncInfo(
                            on_wait=extra[i : i + max_waits], on_update=[]
                        )
                        out_list.append(nd)
                    si.on_wait = keep
                    changed = True
                out_list.append(ins)
            if changed:
                bb.instructions = out_list
    return n_fix


def build_program(apply_waitfix=True):
    nc = bass.Bass("TRN2", target_bir_lowering=False, debug=False)

    xt = nc.dram_tensor("xt", [D, RPC], BF16, kind="ExternalInput").ap()
    mupt = nc.dram_tensor("mupt", [D, K], BF16, kind="ExternalInput").ap()
    bc = nc.dram_tensor("bc", [128, 4], F32, kind="ExternalInput").ap()
    o8 = nc.dram_tensor("o8", [128, 8], BF16, kind="ExternalInput").ap()
    rs_dram = nc.dram_tensor("rs_dram", [NCHUNK // 2, 4, MMW], F32, kind="Internal").ap()
    out = nc.dram_tensor("out", [1, 1], F32, kind="ExternalOutput").ap()

    with tile.TileContext(nc) as tc:
        with ExitStack() as ctx:
            cpool = ctx.enter_context(tc.tile_pool(name="const", bufs=1))
            xpool = ctx.enter_context(tc.tile_pool(name="xt", bufs=1))
            pA = ctx.enter_context(tc.tile_pool(name="pA", bufs=2, space="PSUM"))
            pB = ctx.enter_context(tc.tile_pool(name="pB", bufs=1, space="PSUM"))
            pR = ctx.enter_context(tc.tile_pool(name="pR", bufs=1, space="PSUM"))
            epool = ctx.enter_context(tc.tile_pool(name="e", bufs=3))
            e01pool = ctx.enter_context(tc.tile_pool(name="e01", bufs=2))
            rpool = ctx.enter_context(tc.tile_pool(name="rssb", bufs=2))
            mpool = ctx.enter_context(tc.tile_pool(name="misc", bufs=1))

            # first block's two 512-chunks load in parallel on the two DMA
            # queues (sync + scalar); consts interleave.
            mupt_sb = cpool.tile([D, K], BF16, tag="mupt")
            nc.scalar.dma_start(mupt_sb[:], mupt)

            xt_sb = []
            t0 = xpool.tile([D, XT_LOADS[0]], BF16, tag="xt0")
            nc.sync.dma_start(t0[:], xt[:, 0 : XT_LOADS[0]])
            xt_sb.append((t0, 0, XT_LOADS[0]))
            col = XT_LOADS[0]

            bc_sb = cpool.tile([128, 4], F32, tag="bc")
            nc.scalar.dma_start(bc_sb[:], bc)

            t1 = xpool.tile([D, XT_LOADS[1]], BF16, tag="xt1")
            nc.sync.dma_start(t1[:], xt[:, col : col + XT_LOADS[1]])
            xt_sb.append((t1, col, XT_LOADS[1]))
            col += XT_LOADS[1]

            o8_sb = cpool.tile([128, 8], BF16, tag="o8")
            nc.sync.dma_start(o8_sb[:], o8)

            for li, w in enumerate(XT_LOADS[2:], start=2):
                t = xpool.tile([D, w], BF16, tag=f"xt{li}")
                nc.sync.dma_start(t[:], xt[:, col : col + w])
                xt_sb.append((t, col, w))
                col += w

            def xt_slice(c0, w):
                for t, s, n in xt_sb:
                    if s <= c0 and c0 + w <= s + n:
                        return t[:, c0 - s : c0 - s + w]
                raise AssertionError("chunk crosses load boundary")

            rsd = mpool.tile([128, NCHUNK * 8], F32, tag="rsd")
            lnv = mpool.tile([128, NCHUNK * 8], F32, tag="lnv")
            acc = mpool.tile([128, 1], F32, tag="acc")
            acc2 = mpool.tile([104, 1], F32, tag="acc2")
            lnv2 = mpool.tile([104, MMW], BF16, tag="lnv2")
            out_sb = mpool.tile([1, 1], F32, tag="out")

            # prewarm the Exp ACT table so the first exp skips the load
            warm = mpool.tile([128, 1], F32, tag="warm")
            nc.vector.memset(warm[:], 0.0)
            nc.scalar.activation(warm[:], warm[:], ACTF.Exp)

            # prewarm the PE (HAM clock gate): ~3.4us of dummy matmuls
            wdummy = mpool.tile([128, 8], BF16, tag="wdummy")
            nc.vector.memset(wdummy[:], 0.0)
            wdummy2 = mpool.tile([128, MMW], BF16, tag="wdummy2")
            nc.vector.memset(wdummy2[:], 0.0)
            warm_ps = pA.tile([128, CW], F32, tag="A")
            for _ in range(6):
                nc.tensor.matmul(
                    warm_ps[0:8, 0:MMW],
                    lhsT=wdummy[:],
                    rhs=wdummy2[:],
                    start=True,
                    stop=True,
                )

            rs_batch = [None]

            def rowsum_span(e01_ap, s):
                # global 512-span s -> pR tile s//4, col group s%4
                g = s % 4
                if g == 0:
                    rs = pR.tile([128, MMW], F32, tag="rs")
                    rs_batch[0] = rs
                    if s // 4 == NCHUNK // 2 - 1:
                        # last tile is Ln'd from PSUM; junk rows need ln(1)=0
                        nc.vector.memset(rs[:], 1.0)
                rs = rs_batch[0]
                nc.tensor.matmul(
                    rs[32 * g : 32 * g + 8, :],
                    lhsT=o8_sb[:],
                    rhs=e01_ap,
                    start=True,
                    stop=True,
                    tile_position=(0, 32 * g),
                )
                if g == 3 and s // 4 < NCHUNK // 2 - 1:
                    m = s // 4
                    rssb = rpool.tile([104, MMW], F32, tag="rssb")
                    nc.vector.tensor_copy(rssb[:], rs[0:104, :])
                    nc.sync.dma_start(rs_dram[m], rssb[0:97:32, :])
                    if m % 2 == 1 or m == NCHUNK // 2 - 2:
                        lo = m - 1 if m % 2 == 1 else m
                        src = rs_dram[lo : m + 1].rearrange(
                            "a b (p c) -> (a b p) c", p=4, c=NCHUNK * 8
                        )
                        nc.sync.dma_start(
                            rsd[16 * lo : 16 * (m + 1), :], src
                        )

            def rowsum(e01, j):
                rowsum_span(e01[:, 0:MMW], 2 * j)
                rowsum_span(e01[:, MMW:CW], 2 * j + 1)

            prev = None
            for j in range(NCHUNK):
                psA = pA.tile([128, CW], F32, tag="A")
                psB = pB.tile([128, CW], F32, tag="B")
                for o in range(0, CW, MMW):
                    nc.tensor.matmul(
                        psA[:, o : o + MMW],
                        lhsT=mupt_sb[:, 0:128],
                        rhs=xt_slice(j * CW + o, MMW),
                        start=True,
                        stop=True,
                    )
                for o in range(0, CW, MMW):
                    nc.tensor.matmul(
                        psB[:, o : o + MMW],
                        lhsT=mupt_sb[:, 128:256],
                        rhs=xt_slice(j * CW + o, MMW),
                        start=True,
                        stop=True,
                    )
                if prev is not None:
                    rowsum(*prev)

                if j < NCHUNK - 1:
                    ee = epool.tile([128, 2 * CW], BF16, tag="ee")
                    nc.scalar.activation(
                        ee[:, 0:CW], psA[:], ACTF.Exp, bias=bc_sb[:, 0:1]
                    )
                    nc.scalar.activation(
                        ee[:, CW : 2 * CW], psB[:], ACTF.Exp, bias=bc_sb[:, 1:2]
                    )
                    e01 = e01pool.tile([128, CW], BF16, tag="e01")
                    nc.vector.tensor_add(e01[:], ee[:, 0:CW], ee[:, CW : 2 * CW])
                    prev = (e01, j)
                else:
                    # last block: 512-granular exps/adds so the rowsum spans
                    # and the PSUM-direct Ln start as early as possible
                    e0 = epool.tile([128, CW], BF16, tag="e0")
                    nc.scalar.activation(e0[:], psA[:], ACTF.Exp, bias=bc_sb[:, 0:1])
                    e1a = epool.tile([128, MMW], BF16, tag="e1a")
                    e1b = epool.tile([128, MMW], BF16, tag="e1b")
                    nc.scalar.activation(
                        e1a[:], psB[:, 0:MMW], ACTF.Exp, bias=bc_sb[:, 1:2]
                    )
                    nc.scalar.activation(
                        e1b[:], psB[:, MMW:CW], ACTF.Exp, bias=bc_sb[:, 1:2]
                    )
                    e01a = e01pool.tile([128, MMW], BF16, tag="e01a")
                    e01b = e01pool.tile([128, MMW], BF16, tag="e01b")
                    nc.vector.tensor_add(e01a[:], e0[:, 0:MMW], e1a[:])
                    nc.vector.tensor_add(e01b[:], e0[:, MMW:CW], e1b[:])
                    rowsum_span(e01a[:], 2 * j)
                    rowsum_span(e01b[:], 2 * j + 1)

            rs3 = rs_batch[0]

            # Ln over the respread blocks 0-13 [112,128]; per-partition sums in acc.
            nc.scalar.activation(
                lnv[0:112, :], rsd[0:112, :], ACTF.Ln, accum_out=acc[0:112, 0:1]
            )
            # blocks 14-15: Ln straight from PSUM (8x replicas; junk rows ln(1)=0)
            nc.scalar.activation(
                lnv2[:], rs3[0:104, :], ACTF.Ln, accum_out=acc2[0:104, 0:1]
            )
            # partition-reduce: ones^T@acc1 + 0.125mask^T@acc2 into [1,1] psum
            rs_fin = pR.tile([128, MMW], F32, tag="rs")
            nc.tensor.matmul(
                rs_fin[0:1, 0:1],
                lhsT=bc_sb[0:112, 2:3],
                rhs=acc[0:112, :],
                start=True,
                stop=False,
            )
            nc.tensor.matmul(
                rs_fin[0:1, 0:1],
                lhsT=bc_sb[0:104, 3:4],
                rhs=acc2[0:104, :],
                start=False,
                stop=True,
            )
            nc.vector.tensor_copy(out_sb[:], rs_fin[0:1, 0:1])
            nc.sync.dma_start(out, out_sb[:])

    if apply_waitfix:
        _strip_same_engine_waits(nc)
        _split_excess_waits(nc)
    return nc


def make_in_maps(x, mu, prec):
    import ml_dtypes

    x = np.asarray(x, dtype=np.float32)
    mu = np.asarray(mu, dtype=np.float32)
    prec = np.asarray(prec, dtype=np.float32)
    mupt = np.ascontiguousarray((mu * prec[None, :]).T).astype(ml_dtypes.bfloat16)
    musq_half = 0.5 * ((mu.astype(np.float64) ** 2) @ prec.astype(np.float64))
    bc = np.empty((128, 4), np.float32)
    bc[:, 0] = (CSHIFT - musq_half[0:128]).astype(np.float32)
    bc[:, 1] = (CSHIFT - musq_half[128:256]).astype(np.float32)
    bc[:, 2] = 1.0
    bc[:, 3] = np.where(np.arange(128) % 32 < 8, 0.125, 0.0).astype(np.float32)
    o8 = np.ones((128, 8), np.float32).astype(ml_dtypes.bfloat16)
    in_maps = []
    for c in range(NCORES):
        xt_c = np.ascontiguousarray(x[c * RPC : (c + 1) * RPC, :].T).astype(
            ml_dtypes.bfloat16
        )
        in_maps.append({"xt": xt_c, "mupt": mupt, "bc": bc, "o8": o8})
    return in_maps


def combine_outputs(outs, x, prec):
    x64 = np.asarray(x, dtype=np.float64)
    prec64 = np.asarray(prec, dtype=np.float64)
    s_xx = float(((x64 * x64) @ prec64).sum())
    lse_sum = 0.0
    for o in outs:
        lse_sum += float(np.asarray(o, dtype=np.float64)[0, 0])
    total = 0.5 * s_xx - (lse_sum - N * CSHIFT)
    return np.float32(total)


_CACHED_NC = None


def kernel(x, mu, prec):
    global _CACHED_NC
    if _CACHED_NC is None:
        _CACHED_NC = build_program()
    nc = _CACHED_NC
    in_maps = make_in_maps(x, mu, prec)
    res = run_bass_kernel_spmd(nc, in_maps, core_ids=list(range(NCORES)))
    outs = [res.results[c]["out"] for c in range(NCORES)]
    return combine_outputs(outs, x, prec)


if __name__ == "__main__":
    import reference

    inputs = {k: np.asarray(v) for k, v in reference.setup_inputs().items()}
    expected = float(reference.reference(**inputs))
    actual = float(kernel(**inputs))
    rel = abs(actual - expected) / max(1.0, abs(expected))
    print(f"expected={expected:.6f} actual={actual:.6f} rel={rel:.3e}")


# revision 3
# speedup vs baseline: 1.0212x; 1.0212x over previous
"""Trainium2 Bass kernel for nn_Mixture_24541443129646.

loss = 0.5*S_xx - sum_n log sum_k exp(cross[n,k] - musq[k]/2)   (+ N*C shift)

Transposed layout (k on partitions), data-parallel over N on 8 cores.
ACT-centric pipeline. Per 1024-col block j:
  PE:   psA(j%2) = mupt0^T @ xt_j ; psB = mupt1^T @ xt_j
        rowsum(j-1): o8^T @ e01 -> pR partitions 32*(j-1)%4 (col tile_position)
  ACT:  e0 = Exp(psA + bc0), e1 = Exp(psB + bc1)          (the clock, ~1.04us each)
  DVE:  e01 = e0 + e1 (bf16 perf mode)
v5: rowsums of 4 consecutive blocks pack into ONE [128,1024] pR tile at
partition groups 0/32/64/96, so the PSUM->SBUF copy + DRAM drain happen
once per 4 blocks and the pR reuse cycle has ~4 exp-periods of slack
(v4 lagged: per-block copy->rsMM->copy cycle was 2.7us > 2.36us period).
Tail: ONE reload rs_dram -> rsd [128,128], Ln+accum, out.
PSUM banks: pA bufs=2 (4) + pB bufs=1 (2) + pR bufs=1 (2) = 8.
Host: shard/transpose/cast x, fold prec into mu, S_xx, final scalar.
"""

import sys

sys.path.insert(0, "/opt/trn_rl_repo")

from contextlib import ExitStack

import numpy as np

import concourse.bass as bass
import concourse.tile as tile
from concourse import mybir
from concourse.bass_utils import run_bass_kernel_spmd

N, K, D = 131072, 256, 128
NCORES = 8
RPC = N // NCORES
CW = 1024
NCHUNK = RPC // CW
MMW = 512
CSHIFT = 40.0

F32 = mybir.dt.float32
BF16 = mybir.dt.bfloat16
ACTF = mybir.ActivationFunctionType

XT_LOADS = [1024, 1024, 2048, 4096, 4096, 4096]
assert sum(XT_LOADS) == RPC


def _strip_same_engine_waits(nc):
    """Remove waits on semaphores that are only ever incremented by EARLIER
    non-DMA instructions of the SAME engine. Engines execute their queues in
    order, so these waits are satisfied by construction; leaving them in
    blocks back-to-back issue pipelining (~150ns per exp on ACT).

    Safe here: no engine in this kernel reads data written by an earlier
    instruction of the same engine (all data deps are cross-engine and keep
    their waits)."""
    from collections import defaultdict

    all_ins = []
    for f in nc.m.functions:
        for bb in f.blocks:
            all_ins.extend(bb.instructions)

    STRIP_ENGINES = {
        mybir.EngineType.Activation,
        mybir.EngineType.PE,
        mybir.EngineType.DVE,
    }

    # sem id -> engine whose instructions increment it (None if >1 engine or
    # non-inc update modes touch it). DMA-issued increments post at DMA
    # completion, not instruction order, so they never count toward `cum`;
    # they only disqualify the sem if issued from a DIFFERENT engine.
    sem_ok = {}
    for ins in all_ins:
        si = ins.sync_info
        if si is None:
            continue
        for u in si.on_update:
            sid = u.id
            if u.update_mode != "sem-inc":
                sem_ok[sid] = None
                continue
            if sid not in sem_ok:
                sem_ok[sid] = ins.engine
            elif sem_ok[sid] != ins.engine:
                sem_ok[sid] = None

    n_strip = 0
    cum = defaultdict(int)  # same-engine COMPUTE increments only
    for ins in all_ins:
        si = ins.sync_info
        if si is None:
            continue
        is_dma = "DMA" in type(ins).__name__ or "Dma" in type(ins).__name__
        if si.on_wait:
            keep = []
            for w in si.on_wait:
                if (
                    w.wait_mode == "sem-ge-imm"
                    and ins.engine in STRIP_ENGINES
                    and sem_ok.get(w.id) == ins.engine
                    and cum[w.id] >= w.wait_value
                ):
                    n_strip += 1
                else:
                    keep.append(w)
            si.on_wait = keep
        if not is_dma:
            for u in si.on_update:
                if u.update_mode == "sem-inc":
                    cum[u.id] += u.update_value
    return n_strip


def _split_excess_waits(nc, max_waits=1):
    import bass_rust

    n_fix = 0
    for f in nc.m.functions:
        for bb in f.blocks:
            insts = bb.instructions
            out_list = []
            changed = False
            for ins in insts:
                si = ins.sync_info
                if si is not None and len(si.on_wait) > max_waits:
                    waits = list(si.on_wait)
                    extra, keep = waits[:-max_waits], waits[-max_waits:]
                    for i in range(0, len(extra), max_waits):
                        nd = mybir.InstDrain(name=f"I-waitfix-{n_fix}", ins=[], outs=[])
                        n_fix += 1
                        nd.engine = ins.engine
                        nd.sync_info = bass_rust.SyncInfo(
                            on_wait=extra[i : i + max_waits], on_update=[]
                        )
                        out_list.append(nd)
                    si.on_wait = keep
                    changed = True
                out_list.append(ins)
            if changed:
                bb.instructions = out_list
    return n_fix


def build_program(apply_waitfix=True):
    nc = bass.Bass("TRN2", target_bir_lowering=False, debug=False)

    xt = nc.dram_tensor("xt", [D, RPC], BF16, kind="ExternalInput").ap()
    mupt = nc.dram_tensor("mupt", [D, K], BF16, kind="ExternalInput").ap()
    bc = nc.dram_tensor("bc", [128, 4], F32, kind="ExternalInput").ap()
    o8 = nc.dram_tensor("o8", [128, 8], BF16, kind="ExternalInput").ap()
    rs_dram = nc.dram_tensor("rs_dram", [NCHUNK // 2, 4, MMW], F32, kind="Internal").ap()
    out = nc.dram_tensor("out", [1, 1], F32, kind="ExternalOutput").ap()

    with tile.TileContext(nc) as tc:
        with ExitStack() as ctx:
            cpool = ctx.enter_context(tc.tile_pool(name="const", bufs=1))
            xpool = ctx.enter_context(tc.tile_pool(name="xt", bufs=1))
            pA = ctx.enter_context(tc.tile_pool(name="pA", bufs=2, space="PSUM"))
            pB = ctx.enter_context(tc.tile_pool(name="pB", bufs=1, space="PSUM"))
            pR = ctx.enter_context(tc.tile_pool(name="pR", bufs=1, space="PSUM"))
            epool = ctx.enter_context(tc.tile_pool(name="e", bufs=3))
            e01pool = ctx.enter_context(tc.tile_pool(name="e01", bufs=2))
            rpool = ctx.enter_context(tc.tile_pool(name="rssb", bufs=2))
            mpool = ctx.enter_context(tc.tile_pool(name="misc", bufs=1))

            # first block's two 512-chunks load in parallel on the two DMA
            # queues (sync + scalar); consts interleave.
            mupt_sb = cpool.tile([D, K], BF16, tag="mupt")
            nc.scalar.dma_start(mupt_sb[:], mupt)

            xt_sb = []
            t0 = xpool.tile([D, XT_LOADS[0]], BF16, tag="xt0")
            nc.sync.dma_start(t0[:], xt[:, 0 : XT_LOADS[0]])
            xt_sb.append((t0, 0, XT_LOADS[0]))
            col = XT_LOADS[0]

            bc_sb = cpool.tile([128, 4], F32, tag="bc")
            nc.scalar.dma_start(bc_sb[:], bc)

            t1 = xpool.tile([D, XT_LOADS[1]], BF16, tag="xt1")
            nc.sync.dma_start(t1[:], xt[:, col : col + XT_LOADS[1]])
            xt_sb.append((t1, col, XT_LOADS[1]))
            col += XT_LOADS[1]

            o8_sb = cpool.tile([128, 8], BF16, tag="o8")
            nc.sync.dma_start(o8_sb[:], o8)

            for li, w in enumerate(XT_LOADS[2:], start=2):
                t = xpool.tile([D, w], BF16, tag=f"xt{li}")
                nc.sync.dma_start(t[:], xt[:, col : col + w])
                xt_sb.append((t, col, w))
                col += w

            def xt_slice(c0, w):
                for t, s, n in xt_sb:
                    if s <= c0 and c0 + w <= s + n:
                        return t[:, c0 - s : c0 - s + w]
                raise AssertionError("chunk crosses load boundary")

            rsd = mpool.tile([128, NCHUNK * 8], F32, tag="rsd")
            lnv = mpool.tile([128, NCHUNK * 8], F32, tag="lnv")
            acc = mpool.tile([128, 1], F32, tag="acc")
            acc2 = mpool.tile([104, 1], F32, tag="acc2")
            lnv2 = mpool.tile([104, MMW], BF16, tag="lnv2")
            out_sb = mpool.tile([1, 1], F32, tag="out")

            # prewarm the Exp ACT table so the first exp skips the load
            warm = mpool.tile([128, 1], F32, tag="warm")
            nc.vector.memset(warm[:], 0.0)
            nc.scalar.activation(warm[:], warm[:], ACTF.Exp)

            # prewarm the PE (HAM clock gate): ~3.4us of dummy matmuls
            wdummy = mpool.tile([128, 8], BF16, tag="wdummy")
            nc.vector.memset(wdummy[:], 0.0)
            wdummy2 = mpool.tile([128, MMW], BF16, tag="wdummy2")
            nc.vector.memset(wdummy2[:], 0.0)
            warm_ps = pA.tile([128, CW], F32, tag="A")
            for _ in range(6):
                nc.tensor.matmul(
                    warm_ps[0:8, 0:MMW],
                    lhsT=wdummy[:],
                    rhs=wdummy2[:],
                    start=True,
                    stop=True,
                )

            rs_batch = [None]

            def rowsum_span(e01_ap, s):
                # global 512-span s -> pR tile s//4, col group s%4
                g = s % 4
                if g == 0:
                    rs = pR.tile([128, MMW], F32, tag="rs")
                    rs_batch[0] = rs
                    if s // 4 == NCHUNK // 2 - 1:
                        # last tile is Ln'd from PSUM; junk rows need ln(1)=0
                        nc.vector.memset(rs[:], 1.0)
                rs = rs_batch[0]
                nc.tensor.matmul(
                    rs[32 * g : 32 * g + 8, :],
                    lhsT=o8_sb[:],
                    rhs=e01_ap,
                    start=True,
                    stop=True,
                    tile_position=(0, 32 * g),
                )
                if g == 3 and s // 4 < NCHUNK // 2 - 1:
                    m = s // 4
                    rssb = rpool.tile([104, MMW], F32, tag="rssb")
                    nc.vector.tensor_copy(rssb[:], rs[0:104, :])
                    nc.sync.dma_start(rs_dram[m], rssb[0:97:32, :])
                    if m % 2 == 1 or m == NCHUNK // 2 - 2:
                        lo = m - 1 if m % 2 == 1 else m
                        src = rs_dram[lo : m + 1].rearrange(
                            "a b (p c) -> (a b p) c", p=4, c=NCHUNK * 8
                        )
                        nc.sync.dma_start(
                            rsd[16 * lo : 16 * (m + 1), :], src
                        )

            def rowsum(e01, j):
                rowsum_span(e01[:, 0:MMW], 2 * j)
                rowsum_span(e01[:, MMW:CW], 2 * j + 1)

            prev = None
            for j in range(NCHUNK):
                psA = pA.tile([128, CW], F32, tag="A")
                psB = pB.tile([128, CW], F32, tag="B")
                for o in range(0, CW, MMW):
                    nc.tensor.matmul(
                        psA[:, o : o + MMW],
                        lhsT=mupt_sb[:, 0:128],
                        rhs=xt_slice(j * CW + o, MMW),
                        start=True,
                        stop=True,
                    )
                for o in range(0, CW, MMW):
                    nc.tensor.matmul(
                        psB[:, o : o + MMW],
                        lhsT=mupt_sb[:, 128:256],
                        rhs=xt_slice(j * CW + o, MMW),
                        start=True,
                        stop=True,
                    )
                if prev is not None:
                    rowsum(*prev)

                if j < NCHUNK - 1:
                    ee = epool.tile([128, 2 * CW], BF16, tag="ee")
                    nc.scalar.activation(
                        ee[:, 0:CW], psA[:], ACTF.Exp, bias=bc_sb[:, 0:1]
                    )
                    nc.scalar.activation(
                        ee[:, CW : 2 * CW], psB[:], ACTF.Exp, bias=bc_sb[:, 1:2]
                    )
                    e01 = e01pool.tile([128, CW], BF16, tag="e01")
                    nc.vector.tensor_add(e01[:], ee[:, 0:CW], ee[:, CW : 2 * CW])
                    prev = (e01, j)
                else:
                    # last block: 512-granular exps/adds so the rowsum spans
                    # and the PSUM-direct Ln start as early as possible
                    e0 = epool.tile([128, CW], BF16, tag="e0")
                    nc.scalar.activation(e0[:], psA[:], ACTF.Exp, bias=bc_sb[:, 0:1])
                    e1a = epool.tile([128, MMW], BF16, tag="e1a")
                    e1b = epool.tile([128, MMW], BF16, tag="e1b")
                    nc.scalar.activation(
                        e1a[:], psB[:, 0:MMW], ACTF.Exp, bias=bc_sb[:, 1:2]
                    )
                    nc.scalar.activation(
                        e1b[:], psB[:, MMW:CW], ACTF.Exp, bias=bc_sb[:, 1:2]
                    )
                    e01a = e01pool.tile([128, MMW], BF16, tag="e01a")
                    e01b = e01pool.tile([128, MMW], BF16, tag="e01b")
                    nc.vector.tensor_add(e01a[:], e0[:, 0:MMW], e1a[:])
                    nc.vector.tensor_add(e01b[:], e0[:, MMW:CW], e1b[:])
                    rowsum_span(e01a[:], 2 * j)
                    rowsum_span(e01b[:], 2 * j + 1)

            rs3 = rs_batch[0]

            # Ln over the respread blocks 0-13 [112,128]; per-partition sums in acc.
            nc.scalar.activation(
                lnv[0:112, :], rsd[0:112, :], ACTF.Ln, accum_out=acc[0:112, 0:1]
            )
            # blocks 14-15: Ln straight from PSUM (8x replicas; junk rows ln(1)=0)
            nc.scalar.activation(
                lnv2[:], rs3[0:104, :], ACTF.Ln, accum_out=acc2[0:104, 0:1]
            )
            # partition-reduce: ones^T@acc1 + 0.125mask^T@acc2 into [1,1] psum
            rs_fin = pR.tile([128, MMW], F32, tag="rs")
            nc.tensor.matmul(
                rs_fin[0:1, 0:1],
                lhsT=bc_sb[0:112, 2:3],
                rhs=acc[0:112, :],
                start=True,
                stop=False,
            )
            nc.tensor.matmul(
                rs_fin[0:1, 0:1],
                lhsT=bc_sb[0:104, 3:4],
                rhs=acc2[0:104, :],
                start=False,
                stop=True,
            )
            nc.vector.tensor_copy(out_sb[:], rs_fin[0:1, 0:1])
            nc.sync.dma_start(out, out_sb[:])

    if apply_waitfix:
        _strip_same_engine_waits(nc)
        _split_excess_waits(nc)
    return nc


def make_in_maps(x, mu, prec):
    import ml_dtypes

    x = np.asarray(x, dtype=np.float32)
    mu = np.asarray(mu, dtype=np.float32)
    prec = np.asarray(prec, dtype=np.float32)
    mupt = np.ascontiguousarray((mu * prec[None, :]).T).astype(ml_dtypes.bfloat16)
    musq_half = 0.5 * ((mu.astype(np.float64) ** 2) @ prec.astype(np.float64))
    bc = np.empty((128, 4), np.float32)
    bc[:, 0] = (CSHIFT - musq_half[0:128]).astype(np.float32)
    bc[:, 1] = (CSHIFT - musq_half[128:256]).astype(np.float32)
    bc[:, 2] = 1.0
    bc[:, 3] = np.where(np.arange(128) % 32 < 8, 0.125, 0.0).astype(np.float32)
    o8 = np.ones((128, 8), np.float32).astype(ml_dtypes.bfloat16)
    in_maps = []
    for c in range(NCORES):
        xt_c = np.ascontiguousarray(x[c * RPC : (c + 1) * RPC, :].T).astype(
            ml_dtypes.bfloat16
        )
        in_maps.append({"xt": xt_c, "mupt": mupt, "bc": bc, "o8": o8})
    return in_maps


def combine_outputs(outs, x, prec):
    x64 = np.asarray(x, dtype=np.float64)
    prec64 = np.asarray(prec, dtype=np.float64)
    s_xx = float(((x64 * x64) @ prec64).sum())
    lse_sum = 0.0
    for o in outs:
        lse_sum += float(np.asarray(o, dtype=np.float64)[0, 0])
    total = 0.5 * s_xx - (lse_sum - N * CSHIFT)
    return np.float32(total)


_CACHED_NC = None


def kernel(x, mu, prec):
    global _CACHED_NC
    if _CACHED_NC is None:
        _CACHED_NC = build_program()
    nc = _CACHED_NC
    in_maps = make_in_maps(x, mu, prec)
    res = run_bass_kernel_spmd(nc, in_maps, core_ids=list(range(NCORES)))
    outs = [res.results[c]["out"] for c in range(NCORES)]
    return combine_outputs(outs, x, prec)


if __name__ == "__main__":
    import reference

    inputs = {k: np.asarray(v) for k, v in reference.setup_inputs().items()}
    expected = float(reference.reference(**inputs))
    actual = float(kernel(**inputs))
    rel = abs(actual - expected) / max(1.0, abs(expected))
    print(f"expected={expected:.6f} actual={actual:.6f} rel={rel:.3e}")
